# revision 1
# baseline (speedup 1.0000x reference)
"""DVAE GNN message-passing kernel for 8 Trainium2 NeuronCores.

Data parallel over batch B=2048 -> 256 graphs/core. Each core runs the full
20-step topological scan with all weights replicated.

Math (per sample b, step v in 0..19, Hfwd starts at 0):
  gated_u = sigmoid(Wg @ [H_u, e_u] + bg) * (Wm @ [H_u, e_u])
  Hin_v   = sum_u adj[b,u,v] * gated_u          (u >= v rows of Hfwd are 0,
            so gated_u there is a constant c_u)
  H_v     = GRUCell(x_v, Hin_v)
  mu,lv   = W1 @ H_19 + b1, W2 @ H_19 + b2

Device layout: batch-major activations [128b, feat]; matmuls run with the
activation (transposed via PE) as the stationary operand and weights moving,
so outputs land batch-major in PSUM. Biases and the vertex-id one-hot
contributions are folded into the matmuls via ones-rows / one-hot k-chunks.
The adj-weighted message sum runs as fused per-partition-scalar MACs
(scalar_tensor_tensor) split across DVE (batch tile 0) and GPSIMD (tile 1);
the constant part (u >= v) is a real matmul over the u axis seeding the
accumulator in PSUM.
"""

import sys
import numpy as np

for _p in ("/opt/trn_rl_repo",):
    if _p not in sys.path:
        sys.path.insert(0, _p)

B, MAXN, NVT, HS, NZ = 2048, 20, 26, 501, 56
HS2 = HS + 1                  # 502: fp32r needs even innermost free counts
NVT_EFF = NVT + MAXN          # 46
XDIM = NVT_EFF + 1            # 47
NCORES = 8
BS = B // NCORES              # 256 samples per core
G3 = 3 * HS                   # 1503
RZ = 2 * HS                   # 1002

# k-chunking of the augmented hidden axis (501 rows of H^T + ones row)
CH = [(0, 128), (128, 128), (256, 128), (384, 118)]  # covers 0..501 inclusive
# gated-side chunks: + vid one-hot rows appended (total 522 rows)
CHG = [(0, 128), (128, 128), (256, 128), (384, 128), (512, 10)]
CHH = [(0, 128), (128, 128), (256, 128), (384, 128), (512, 10)]  # H^T tile shapes
# transpose source column ranges (chunk3 includes the ones column at HS)
TCH = [(0, 128), (128, 128), (256, 128), (384, 118)]

MM_DTYPE = "f32r"  # "f32r" (1 cyc/row, tf32-ish) | "f32" (4 cyc/row, exact)
USE_GPSIMD = True  # False: route all elementwise TT ops to DVE
ABLATE_ROWDMA = False  # drop per-step ones/vid row DMAs (timing experiment)
ABLATE_CHAIN = False   # drop mask-sum chains (timing experiment)
ABLATE_GATED = False   # drop gated Z/M matmuls + G production (timing)
ABLATE_GRUMM = False   # drop rz/hn/in matmuls (timing)
REPEAT = 1             # repeat the whole computation in-NEFF (timing experiment)
CHAIN2OP = True        # chain as tsmul+TTadd instead of fused stt
PSUM_UNIFIED = False   # one shared 8-buf psum pool instead of 3 pools
WORK_BUFS = 1          # bufs for r/z/tmp/n work tiles


def _pack_layout():
    """Column layout (fp32 elements) of the single packed static tensor.

    Returns (entries, ncols); entries: name -> (row0, nrows, col0, ncols).
    All matmul-consumed slices start at partition 0 or 64.
    """
    ents = {}
    col = 0

    def put(name, row0, nrows, ncols):
        nonlocal col
        ents[name] = (row0, nrows, col, ncols)
        col += ncols

    put("pk", 0, 84, MAXN * BS)          # rows 0:48 X^T+ones, 64:84 adjT masked
    for i, (o, s) in enumerate(CH):
        put(f"wrzh{i}", 0, s, 2 * HS2)
    for i, (o, s) in enumerate(CH):
        put(f"whn{i}", 0, s, HS2)
    put("wrzx", 0, XDIM + 1, 2 * HS2)
    put("wxnc", 0, 84, HS2)              # rows 0:48 W_in^T+bias, 64:84 C
    for i, (o, s) in enumerate(CH):
        put(f"wg{i}", 0, s, HS2)
    put("wgv", 0, MAXN, HS2)
    for i, (o, s) in enumerate(CH):
        put(f"wm{i}", 0, s, HS2)
    put("wmv", 0, MAXN, HS2)
    put("eye20", 0, MAXN, MAXN)
    for i, (o, s) in enumerate(CH):
        put(f"w12{i}", 0, s, 2 * NZ)
    put("adjg0", 0, 128, MAXN * MAXN)
    put("adjg1", 0, 128, MAXN * MAXN)
    put("ident", 0, 128, 128)
    return ents, col


_PROG = None  # cached Bass program


def _build_program():
    import concourse.bass as bass
    import concourse.tile as tile
    from concourse import bacc, mybir

    f32 = mybir.dt.float32
    f32r = mybir.dt.float32r
    mdt = {"f32r": f32r, "f32": f32, "bf16": mybir.dt.bfloat16}[MM_DTYPE]
    AF = mybir.ActivationFunctionType
    OP = mybir.AluOpType

    nc = bacc.Bacc("TRN2", target_bir_lowering=False, debug=False)

    def din(name, shape, dt=None):
        return nc.dram_tensor(name, shape, dt or mdt, kind="ExternalInput").ap()

    ents, ncols = _pack_layout()
    d_wpack = din("wpack", [128, ncols])

    d_out = nc.dram_tensor("out", [BS, 2 * NZ], f32, kind="ExternalOutput").ap()

    def mm(out, lhsT, rhs, start, stop):
        nc.tensor.matmul(out, lhsT, rhs, start=start, stop=stop)

    with tile.TileContext(nc) as tc:
        with (
            tc.tile_pool(name="statics", bufs=1) as sp,
            tc.tile_pool(name="gstore", bufs=2 * (MAXN - 1)) as gp,
            tc.tile_pool(name="hint", bufs=2) as hip,
            tc.tile_pool(name="ht", bufs=2) as htp,
            tc.tile_pool(name="work1", bufs=WORK_BUFS) as wp1,
            tc.tile_pool(name="work2", bufs=2) as wp2,
            tc.tile_pool(name="pp_rz", bufs=(8 if PSUM_UNIFIED else 3),
                         space="PSUM") as pp_rz,
            tc.tile_pool(name="pp_tps", bufs=3, space="PSUM") as _pp_tps,
            tc.tile_pool(name="pp_hn", bufs=2, space="PSUM") as _pp_hn,
        ):
            pp_tps = pp_rz if PSUM_UNIFIED else _pp_tps
            pp_hn = pp_rz if PSUM_UNIFIED else _pp_hn
            # ---- one packed static load: a single DMA -> a single wait sem ----
            WPACK = sp.tile([128, ncols], mdt, tag="wpack", name="wpack")
            nc.sync.dma_start(WPACK[:, :], d_wpack)

            def sl(name, dt=None):
                r0, nr, c0, ncl = ents[name]
                ap = WPACK[r0:r0 + nr, c0:c0 + ncl]
                return ap.bitcast(dt) if dt else ap

            PK = sl("pk")
            WRZH = [sl(f"wrzh{i}") for i in range(4)]
            WHN = [sl(f"whn{i}") for i in range(4)]
            WRZX = sl("wrzx")
            WXNC = sl("wxnc")
            WG = [sl(f"wg{i}") for i in range(4)]
            WM = [sl(f"wm{i}") for i in range(4)]
            WGV, WMV, EYE = sl("wgv"), sl("wmv"), sl("eye20")
            W12 = [sl(f"w12{i}") for i in range(4)]
            IDN = sl("ident", f32)
            bf16 = mybir.dt.bfloat16
            ADJG = [sl(f"adjg{t}", f32) for t in range(2)]

            # G storage: gated vectors per (vertex, batch-tile), bf16 so the
            # message chains run in the DVE 2x mode
            Gt = [[gp.tile([128, HS2], bf16, tag="g", name=f"g{_u}_{_t}")
                   for _t in range(2)] for _u in range(MAXN - 1)]

            gpe = nc.gpsimd if USE_GPSIMD else nc.vector

            HT_final = None
            for _rep in range(REPEAT):
              for v in range(MAXN):
                  # ---- message input Hin_v, batch-major, per batch tile ----
                  # acc column HS holds 1.0 (ones row of Hinaug^T after transpose)
                  acc = []
                  for t in range(2):
                      dps = pp_tps.tile([128, 512], f32, tag=("rz" if PSUM_UNIFIED else "tps"), name=f"dps{v}_{t}")
                      # constant part: sum_{u>=v} adj[b,u,v] * C[u]
                      mm(dps[:, :HS2], PK[64:84, v * BS + t * 128:v * BS + (t + 1) * 128],
                         WXNC[64:84, :], start=True, stop=True)
                      a = wp2.tile([128, HS2], f32, tag=f"acc{t}", name=f"acc{v}_{t}")
                      if v == 0 or ABLATE_CHAIN:
                          nc.scalar.copy(a[:, :], dps[:, :HS2])
                      else:
                          # bf16 chain on DVE: 4x mul, 2x fused MACs, f32 tail
                          ab = wp1.tile([128, HS2], bf16, tag=f"accb{t}",
                                        name=f"accb{v}_{t}")
                          ab2 = (wp1.tile([128, HS2], bf16, tag=f"accc{t}",
                                          name=f"accc{v}_{t}") if CHAIN2OP else None)
                          for u in range(v):
                              sc = ADJG[t][:, u * MAXN + v:u * MAXN + v + 1]
                              if u == 0:
                                  nc.vector.tensor_scalar_mul(ab[:, :], Gt[u][t][:, :], sc)
                              elif CHAIN2OP:
                                  nc.vector.tensor_scalar_mul(ab2[:, :], Gt[u][t][:, :], sc)
                                  nc.vector.tensor_tensor(ab[:, :], ab[:, :], ab2[:, :],
                                                          OP.add)
                              else:
                                  nc.vector.scalar_tensor_tensor(
                                      ab[:, :], Gt[u][t][:, :], sc, ab[:, :],
                                      OP.mult, OP.add)
                          nc.vector.tensor_tensor(a[:, :], ab[:, :], dps[:, :HS2], OP.add)
                      nc.gpsimd.memset(a[:, HS:HS2], 1.0)   # ones col -> bias row
                      acc.append(a)

                  # ---- transpose Hin -> Hinaug^T chunk-pair tiles ----
                  # pair tile p holds chunks 2p (cols 0:256) and 2p+1 (cols 256:512)
                  HINT = [hip.tile([128, 512], mdt, tag=f"hint{p}", name=f"hint{v}_{p}")
                          for p in range(2)]
                  for p in range(2):
                      tp = pp_tps.tile([128, 512], f32, tag=("rz" if PSUM_UNIFIED else "tps"), name=f"tpi{v}_{p}")
                      for j in range(2):
                          i = 2 * p + j
                          o, w = TCH[i]
                          for t in range(2):
                              nc.tensor.transpose(
                                  tp[:w, j * 256 + t * 128:j * 256 + (t + 1) * 128],
                                  acc[t][:, o:o + w], IDN[:, :])
                      nc.scalar.copy(HINT[p][:, :], tp[:, :])

                  def hsl(i, t):
                      return HINT[i // 2][0:TCH[i][1], (i % 2) * 256 + t * 128:
                                          (i % 2) * 256 + (t + 1) * 128]

                  # ---- gate matmuls ----
                  rzp, hnp, inp = [], [], []
                  for t in range(2):
                      xsl = PK[0:XDIM + 1, v * BS + t * 128:v * BS + (t + 1) * 128]
                      if ABLATE_GRUMM:
                          ps0 = pp_rz.tile([128, 512], f32, tag="rz", name=f"rz{v}_{t}_0")
                          mm(ps0[:, :HS2], xsl, WRZX[:, 0:HS2], start=True, stop=True)
                          ps1 = pp_rz.tile([128, 512], f32, tag="rz", name=f"rz{v}_{t}_1")
                          mm(ps1[:, :HS2], xsl, WRZX[:, HS2:2 * HS2], start=True, stop=True)
                          rzp += [ps0, ps1]
                          hnx = pp_hn.tile([128, 512], f32, tag=("rz" if PSUM_UNIFIED else "hn"), name=f"hn{v}_{t}")
                          mm(hnx[:, :HS2], xsl, WXNC[0:XDIM + 1, :], start=True, stop=True)
                          hnp.append(hnx)
                          ipx = pp_tps.tile([128, 512], f32, tag=("rz" if PSUM_UNIFIED else "tps"), name=f"in{v}_{t}")
                          mm(ipx[:, :HS2], xsl, WXNC[0:XDIM + 1, :], start=True, stop=True)
                          inp.append(ipx)
                          continue
                      for j in range(2):  # r and z halves
                          ps = pp_rz.tile([128, 512], f32, tag="rz", name=f"rz{v}_{t}_{j}")
                          for i in range(4):
                              mm(ps[:, :HS2], hsl(i, t),
                                 WRZH[i][:, j * HS2:(j + 1) * HS2], start=(i == 0), stop=False)
                          mm(ps[:, :HS2], xsl, WRZX[:, j * HS2:(j + 1) * HS2],
                             start=False, stop=True)
                          rzp.append(ps)
                      hn = pp_hn.tile([128, 512], f32, tag=("rz" if PSUM_UNIFIED else "hn"), name=f"hn{v}_{t}")
                      for i in range(4):
                          mm(hn[:, :HS2], hsl(i, t), WHN[i][:, :],
                             start=(i == 0), stop=(i == 3))
                      hnp.append(hn)
                      ip = pp_tps.tile([128, 512], f32, tag=("rz" if PSUM_UNIFIED else "tps"), name=f"in{v}_{t}")
                      mm(ip[:, :HS2], xsl, WXNC[0:XDIM + 1, :], start=True, stop=True)
                      inp.append(ip)

                  # ---- GRU elementwise ----
                  hb = []
                  for t in range(2):
                      r = wp1.tile([128, HS2], f32, tag=f"r{t}", name=f"r{v}_{t}")
                      z = wp1.tile([128, HS2], f32, tag=f"z{t}", name=f"z{v}_{t}")
                      nc.scalar.activation(r[:, :HS], rzp[2 * t][:, :HS], AF.Sigmoid)
                      nc.scalar.activation(z[:, :HS], rzp[2 * t + 1][:, :HS], AF.Sigmoid)
                      tmp = wp1.tile([128, HS], f32, tag=f"tmp{t}", name=f"tmp{v}_{t}")
                      nc.vector.tensor_tensor(tmp[:, :], r[:, :HS], hnp[t][:, :HS], OP.mult)
                      nc.vector.tensor_tensor(tmp[:, :], tmp[:, :], inp[t][:, :HS], OP.add)
                      n = wp1.tile([128, HS], f32, tag=f"n{t}", name=f"n{v}_{t}")
                      nc.scalar.activation(n[:, :], tmp[:, :], AF.Tanh)
                      d = wp1.tile([128, HS], f32, tag=f"tmp{t}", name=f"d{v}_{t}")
                      gpe.tensor_sub(d[:, :], acc[t][:, :HS], n[:, :])
                      h = wp2.tile([128, HS2], f32, tag=f"h{t}", name=f"h{v}_{t}")
                      gpe.tensor_tensor(h[:, :HS], d[:, :], z[:, :HS], OP.mult)
                      gpe.tensor_tensor(h[:, :HS], h[:, :HS], n[:, :], OP.add)
                      nc.gpsimd.memset(h[:, HS:HS2], 1.0)   # ones col -> bg row
                      hb.append(h)

                  # ---- transpose H -> Haug^T chunk-pair tiles ----
                  HT = [htp.tile([128, 512], mdt, tag=f"ht{p}", name=f"ht{v}_{p}")
                        for p in range(2)]
                  for p in range(2):
                      tp = pp_tps.tile([128, 512], f32, tag=("rz" if PSUM_UNIFIED else "tps"), name=f"tph{v}_{p}")
                      for j in range(2):
                          i = 2 * p + j
                          o, w = TCH[i]
                          for t in range(2):
                              nc.tensor.transpose(
                                  tp[:w, j * 256 + t * 128:j * 256 + (t + 1) * 128],
                                  hb[t][:, o:o + w], IDN[:, :])
                      nc.scalar.copy(HT[p][:, :], tp[:, :])

                  def htl(i, t):
                      return HT[i // 2][0:TCH[i][1], (i % 2) * 256 + t * 128:
                                        (i % 2) * 256 + (t + 1) * 128]

                  if v < MAXN - 1 and not ABLATE_GATED:
                      # ---- gated message for this vertex ----
                      # vid one-hot contribution: broadcast-selected column of EYE
                      vsel = EYE[:, v:v + 1].broadcast_to([MAXN, 128])
                      for t in range(2):
                          zp = pp_rz.tile([128, 512], f32, tag="rz", name=f"zp{v}_{t}")
                          mp = pp_rz.tile([128, 512], f32, tag="rz", name=f"mp{v}_{t}")
                          for i in range(4):
                              mm(zp[:, :HS2], htl(i, t), WG[i][:, :],
                                 start=(i == 0), stop=False)
                          mm(zp[:, :HS2], vsel, WGV[:, :], start=False, stop=True)
                          for i in range(4):
                              mm(mp[:, :HS2], htl(i, t), WM[i][:, :],
                                 start=(i == 0), stop=False)
                          mm(mp[:, :HS2], vsel, WMV[:, :], start=False, stop=True)
                          sg = wp1.tile([128, HS2], f32, tag=f"r{t}", name=f"sg{v}_{t}")
                          nc.scalar.activation(sg[:, :], zp[:, :HS2], AF.Sigmoid)
                          mb = wp1.tile([128, HS2], f32, tag=f"z{t}", name=f"mb{v}_{t}")
                          nc.scalar.copy(mb[:, :], mp[:, :HS2])
                          gpe.tensor_tensor(Gt[v][t][:, :], sg[:, :], mb[:, :], OP.mult)
                  if v == MAXN - 1:
                      HT_final = HT

            # ---- readout ----
            HTf = HT_final
            for t in range(2):
                op = pp_hn.tile([128, 512], f32, tag=("rz" if PSUM_UNIFIED else "hn"), name=f"op{t}")
                for i in range(4):
                    ksl = HTf[i // 2][0:TCH[i][1], (i % 2) * 256 + t * 128:
                                      (i % 2) * 256 + (t + 1) * 128]
                    mm(op[:, :2 * NZ], ksl,
                       W12[i][:, :], start=(i == 0), stop=(i == 3))
                ob = wp1.tile([128, 2 * NZ], f32, tag=f"ob{t}", name=f"ob{t}")
                nc.scalar.copy(ob[:, :], op[:, :2 * NZ])
                nc.sync.dma_start(d_out[t * 128:(t + 1) * 128, :], ob[:, :])

    nc.compile()
    return nc


def _host_prep(types, feats, adj, Wg, bg, Wm, W_ih, b_ih, W_hh, b_hh, W1, b1, W2, b2):
    """Build per-core input maps (numpy only)."""
    f = np.float32
    types = np.asarray(types).astype(np.int64)
    feats = np.asarray(feats, dtype=f)
    adj = np.asarray(adj, dtype=f)
    Wg, bg, Wm = np.asarray(Wg, f), np.asarray(bg, f), np.asarray(Wm, f)
    W_ih, b_ih = np.asarray(W_ih, f), np.asarray(b_ih, f)
    W_hh, b_hh = np.asarray(W_hh, f), np.asarray(b_hh, f)
    W1, b1 = np.asarray(W1, f), np.asarray(b1, f)
    W2, b2 = np.asarray(W2, f), np.asarray(b2, f)

    bsz = types.shape[0]
    ncore = NCORES
    bs = bsz // ncore

    # X^T with ones row: [48, MAXN*bs] per core
    X = np.zeros((bsz, MAXN, XDIM + 1), dtype=f)
    onehot = np.eye(NVT_EFF, dtype=f)[types.reshape(-1) % NVT_EFF]
    X[:, :, :NVT_EFF] = onehot.reshape(bsz, MAXN, NVT_EFF)
    X[:, :, NVT_EFF] = feats
    X[:, :, XDIM] = 1.0

    # constant gated vectors c_u for zero hidden state
    zg = 1.0 / (1.0 + np.exp(-(bg[None, :] + Wg[:, HS:].T)))   # [20, 501]
    C = (zg * Wm[:, HS:].T).astype(f)

    def aug(wT, brow):
        return np.concatenate([wT, brow[None, :]], axis=0).astype(f)

    def pad_rz(a):          # [s, 1002] -> [s, 1004] with per-gate 502 halves
        o = np.zeros((a.shape[0], 2 * HS2), dtype=f)
        o[:, :HS] = a[:, :HS]
        o[:, HS2:HS2 + HS] = a[:, HS:]
        return o

    def pad_h(a):           # [s, 501] -> [s, 502]
        o = np.zeros((a.shape[0], HS2), dtype=f)
        o[:, :HS] = a
        return o

    wrzh = pad_rz(aug(W_hh[:RZ].T, b_hh[:RZ]))
    whn = pad_h(aug(W_hh[RZ:].T, b_hh[RZ:]))
    wrzx = pad_rz(aug(W_ih[:RZ].T, b_ih[:RZ]))
    wxnc = np.zeros((84, HS2), dtype=f)
    wxnc[:XDIM + 1] = pad_h(aug(W_ih[RZ:].T, b_ih[RZ:]))
    wxnc[64:84] = pad_h(C)
    wg = pad_h(np.concatenate([Wg[:, :HS].T, bg[None, :]], axis=0).astype(f))
    wgv = pad_h(np.ascontiguousarray(Wg[:, HS:].T))
    wm = pad_h(np.concatenate([Wm[:, :HS].T, np.zeros((1, HS), f)], axis=0))
    wmv = pad_h(np.ascontiguousarray(Wm[:, HS:].T))
    eye20 = np.eye(MAXN, dtype=f)
    w12 = np.concatenate([np.concatenate([W1.T, W2.T], axis=1),
                          np.concatenate([b1, b2])[None, :]], axis=0).astype(f)
    ident = np.eye(128, dtype=f)

    ents, ncols = _pack_layout()

    def place(pack, name, arr):
        r0, nr, c0, ncl = ents[name]
        assert arr.shape == (nr, ncl), (name, arr.shape, (nr, ncl))
        pack[r0:r0 + nr, c0:c0 + ncl] = arr

    umask = (np.arange(MAXN)[:, None] >= np.arange(MAXN)[None, :]).astype(f)

    in_maps = []
    for c in range(ncore):
        sl = slice(c * bs, (c + 1) * bs)
        Xc = X[sl]                                    # [bs, 20, 48]
        xt = Xc.transpose(2, 1, 0).reshape(XDIM + 1, MAXN * bs)
        adjc = adj[sl]                                # [bs, 20, 20]
        # adjT[u, v*bs+b] = adj[b,u,v], zeroed where u < v (only u>=v used)
        adjm = adjc.transpose(1, 2, 0) * umask[:, :, None]
        pk = np.zeros((84, MAXN * bs), dtype=f)
        pk[:XDIM + 1] = xt
        pk[64:84] = adjm.reshape(MAXN, MAXN * bs)

        pack = np.zeros((128, ncols), dtype=f)
        place(pack, "pk", pk)
        for i, (o, s) in enumerate(CH):
            place(pack, f"wrzh{i}", wrzh[o:o + s])
            place(pack, f"whn{i}", whn[o:o + s])
            place(pack, f"w12{i}", w12[o:o + s])
        place(pack, "wrzx", wrzx)
        place(pack, "wxnc", wxnc)
        for i, (o, s) in enumerate(CH):
            place(pack, f"wg{i}", wg[o:o + s])
            place(pack, f"wm{i}", wm[o:o + s])
        place(pack, "wgv", wgv)
        place(pack, "wmv", wmv)
        place(pack, "eye20", eye20)
        adjg = adjc.reshape(bs, MAXN * MAXN)
        place(pack, "adjg0", adjg[:128])
        place(pack, "adjg1", adjg[128:])
        place(pack, "ident", ident)
        in_maps.append(dict(wpack=pack))
    return in_maps


def _get_prog():
    global _PROG
    if _PROG is None:
        _PROG = _build_program()
    return _PROG


def kernel(**inputs):
    from concourse.bass_utils import run_bass_kernel_spmd
    nc = _get_prog()
    in_maps = _host_prep(**inputs)
    res = run_bass_kernel_spmd(nc, in_maps, core_ids=list(range(NCORES)))
    out = np.concatenate([r["out"] for r in res.results], axis=0)
    mu = np.ascontiguousarray(out[:, :NZ])
    logvar = np.ascontiguousarray(out[:, NZ:])
    return mu, logvar



# revision 2
# speedup vs baseline: 1.1956x; 1.1956x over previous
"""DVAE GNN message-passing kernel for 8 Trainium2 NeuronCores.

Data parallel over batch B=2048 -> 256 graphs/core (2 partition tiles of
128). Each core runs the full 20-step topological scan, weights replicated.

Math (per sample b, step v in 0..19, Hfwd starts at 0):
  gated_u = sigmoid(Wg @ [H_u, e_u] + bg) * (Wm @ [H_u, e_u])
  Hin_v   = sum_u adj[b,u,v] * gated_u      (u >= v rows: constant C_u)
  H_v     = GRUCell(x_v, Hin_v)
  mu,lv   = W1 @ H_19 + b1, W2 @ H_19 + b2

Structure vs the naive scan:
  - Lazy message accumulators: A_w (bf16, SBUF) holds
    dps_w + sum_{u<done} adj[b,u,w]*G_u.  The constant part (dps_w, a
    matmul over the masked adj rows) is computed for all 20 vertices in a
    prologue that also warms up the PE clock.  After G_v is produced, one
    fused scalar_tensor_tensor per future vertex updates A_w; only the
    w=v+1 update is on the critical path.
  - Activations stay batch-major [128b, feat]; matmul stationaries are
    PE-transposed activation chunks (bf16 transposes, 1 cyc/row, packed
    into a single PSUM bank per tile then copied once to SBUF).
  - The two batch tiles are independent chains; PE work is emitted so one
    tile's matmuls run while the other tile's GRU elementwise phase is on
    DVE/ACT/GPSIMD, keeping the PE continuously busy (full 2.4 GHz clock).
"""

import sys
import numpy as np

for _p in ("/opt/trn_rl_repo",):
    if _p not in sys.path:
        sys.path.insert(0, _p)

B, MAXN, NVT, HS, NZ = 2048, 20, 26, 501, 56
HS2 = HS + 1                  # 502 (even innermost counts for 2x DVE mode)
NVT_EFF = NVT + MAXN          # 46
XDIM = NVT_EFF + 1            # 47
NCORES = 8
BS = B // NCORES              # 256 samples per core
G3 = 3 * HS                   # 1503
RZ = 2 * HS                   # 1002

# k-chunking of the augmented hidden axis (501 rows + ones row at 501)
CH = [(0, 128), (128, 128), (256, 128), (384, 118)]


def _pack_layout():
    """Column layout (fp32 cells) of the packed static tensor, ordered by
    first use so the DMA can be split.  name -> (row0, nrows, col0, ncols)."""
    ents = {}
    col = 0

    def put(name, row0, nrows, ncols):
        nonlocal col
        ents[name] = (row0, nrows, col, ncols)
        col += ncols

    # -- DMA 1: prologue needs (dps matmuls) + step-0 transposes --
    put("pk", 0, 84, MAXN * BS)          # rows 0:48 X^T+ones, 64:84 adjT masked
    put("wxnc", 0, 84, HS2)              # rows 0:48 W_in^T+bias, 64:84 C
    put("identb", 0, 128, 64)            # bf16 identity (bitcast)
    d1 = col
    # -- DMA 2: step loop weights --
    for i, (o, s) in enumerate(CH):
        put(f"wrzh{i}", 0, s, 2 * HS2)
    for i, (o, s) in enumerate(CH):
        put(f"whn{i}", 0, s, HS2)
    put("wrzx", 0, XDIM + 1, 2 * HS2)
    put("adjg0", 0, 128, MAXN * MAXN)
    put("adjg1", 0, 128, MAXN * MAXN)
    for i, (o, s) in enumerate(CH):
        put(f"wg{i}", 0, s, HS2)
    put("wgv", 0, MAXN, HS2)
    for i, (o, s) in enumerate(CH):
        put(f"wm{i}", 0, s, HS2)
    put("wmv", 0, MAXN, HS2)
    put("eye20", 0, MAXN, MAXN)
    d2 = col
    # -- DMA 3: readout --
    for i, (o, s) in enumerate(CH):
        put(f"w12{i}", 0, s, 2 * NZ)
    return ents, col, (d1, d2)


_PROG = None  # cached Bass program


def _build_program():
    import concourse.bass as bass
    import concourse.tile as tile
    from concourse import bacc, mybir

    f32 = mybir.dt.float32
    f32r = mybir.dt.float32r
    bf16 = mybir.dt.bfloat16
    AF = mybir.ActivationFunctionType
    OP = mybir.AluOpType

    nc = bacc.Bacc("TRN2", target_bir_lowering=False, debug=False)

    ents, ncols, (d1, d2) = _pack_layout()
    d_wpack = nc.dram_tensor("wpack", [128, ncols], f32r,
                             kind="ExternalInput").ap()
    d_out = nc.dram_tensor("out", [BS, 2 * NZ], f32, kind="ExternalOutput").ap()

    def mm(out, lhsT, rhs, start, stop):
        nc.tensor.matmul(out, lhsT, rhs, start=start, stop=stop)

    with tile.TileContext(nc) as tc:
        with (
            tc.tile_pool(name="statics", bufs=1) as sp,
            tc.tile_pool(name="apool", bufs=1) as apl,
            tc.tile_pool(name="hg", bufs=1) as hgp,
            tc.tile_pool(name="hint", bufs=2) as hip,
            tc.tile_pool(name="work", bufs=1) as wp,
            tc.tile_pool(name="ps", bufs=8, space="PSUM") as pp,
        ):
            WPACK = sp.tile([128, ncols], f32r, tag="wpack", name="wpack")
            nc.sync.dma_start(WPACK[:, :d1], d_wpack[:, :d1])
            nc.sync.dma_start(WPACK[:, d1:d2], d_wpack[:, d1:d2])
            nc.sync.dma_start(WPACK[:, d2:], d_wpack[:, d2:])

            def sl(name, dt=None):
                r0, nr, c0, ncl = ents[name]
                ap = WPACK[r0:r0 + nr, c0:c0 + ncl]
                return ap.bitcast(dt) if dt else ap

            PK = sl("pk")
            WRZH = [sl(f"wrzh{i}") for i in range(4)]
            WHN = [sl(f"whn{i}") for i in range(4)]
            WRZX = sl("wrzx")
            WXNC = sl("wxnc")
            WG = [sl(f"wg{i}") for i in range(4)]
            WM = [sl(f"wm{i}") for i in range(4)]
            WGV, WMV, EYE = sl("wgv"), sl("wmv"), sl("eye20")
            W12 = [sl(f"w12{i}") for i in range(4)]
            IDB = sl("identb", bf16)            # [128,128] bf16 identity
            ADJG = [sl(f"adjg{t}", f32) for t in range(2)]

            # persistent state: accumulators A[w][t], h[t], G[t]
            A = [[apl.tile([128, HS2], bf16, tag=f"A{w}_{t}", name=f"A{w}_{t}")
                  for t in range(2)] for w in range(MAXN)]
            Ht = [hgp.tile([128, HS2], bf16, tag=f"h{t}", name=f"h{t}")
                  for t in range(2)]
            Gt = [hgp.tile([128, HS2], bf16, tag=f"G{t}", name=f"G{t}")
                  for t in range(2)]

            # ---- prologue: dps_w = sum_{u>=w} adj[b,u,w] * C[u] -> A init ----
            for w in range(MAXN):
                for t in range(2):
                    dps = pp.tile([128, 512], f32, tag="ps", name=f"dps{w}_{t}")
                    mm(dps[:, :HS2],
                       PK[64:84, w * BS + t * 128:w * BS + (t + 1) * 128],
                       WXNC[64:84, :], start=True, stop=True)
                    eng = nc.scalar if (w + t) % 2 == 0 else nc.vector
                    if eng is nc.scalar:
                        eng.copy(A[w][t][:, :HS], dps[:, :HS])
                    else:
                        eng.tensor_copy(A[w][t][:, :HS], dps[:, :HS])
                    nc.gpsimd.memset(A[w][t][:, HS:HS2], 1.0)  # bias ones col
            for t in range(2):
                nc.gpsimd.memset(Ht[t][:, HS:HS2], 1.0)

            HT_final = [None, None]

            # ---- topological scan ----
            for v in range(MAXN):
                # per-tile pipeline stages; emitted tile-staggered so tile1's
                # PE work overlaps tile0's elementwise phase
                inp, hint, rzr, rzz, hnp = [None] * 2, [None] * 2, [None] * 2, [None] * 2, [None] * 2
                r_, z_, tmp_, n_ = [None] * 2, [None] * 2, [None] * 2, [None] * 2
                htt, zpp, mpp, sg_ = [None] * 2, [None] * 2, [None] * 2, [None] * 2

                def stage_in(t):
                    ip = pp.tile([128, 512], f32, tag="ps", name=f"in{v}_{t}")
                    xsl = PK[0:XDIM + 1, v * BS + t * 128:v * BS + (t + 1) * 128]
                    mm(ip[:, :HS2], xsl, WXNC[0:XDIM + 1, :], start=True, stop=True)
                    inp[t] = ip

                def stage_ta(t):
                    ta = pp.tile([128, 512], bf16, tag="ps", name=f"ta{v}_{t}")
                    for i, (o, wd) in enumerate(CH):
                        nc.tensor.transpose(ta[:wd, i * 128:(i + 1) * 128],
                                            A[v][t][:, o:o + wd], IDB[:, :])
                    hi = hip.tile([128, 512], f32r, tag=f"hint{t}",
                                  name=f"hint{v}_{t}")
                    nc.scalar.copy(hi[:, :], ta[:, :])
                    hint[t] = hi

                def stage_gates(t):
                    xsl = PK[0:XDIM + 1, v * BS + t * 128:v * BS + (t + 1) * 128]
                    pr = pp.tile([128, 512], f32, tag="ps", name=f"rzr{v}_{t}")
                    pz = pp.tile([128, 512], f32, tag="ps", name=f"rzz{v}_{t}")
                    ph = pp.tile([128, 512], f32, tag="ps", name=f"hn{v}_{t}")
                    for i, (o, wd) in enumerate(CH):
                        hsl = hint[t][0:wd, i * 128:(i + 1) * 128]
                        mm(pr[:, :HS2], hsl, WRZH[i][:, 0:HS2],
                           start=(i == 0), stop=False)
                        mm(pz[:, :HS2], hsl, WRZH[i][:, HS2:2 * HS2],
                           start=(i == 0), stop=False)
                        mm(ph[:, :HS2], hsl, WHN[i][:, :],
                           start=(i == 0), stop=(i == 3))
                    mm(pr[:, :HS2], xsl, WRZX[:, 0:HS2], start=False, stop=True)
                    mm(pz[:, :HS2], xsl, WRZX[:, HS2:2 * HS2], start=False,
                       stop=True)
                    rzr[t], rzz[t], hnp[t] = pr, pz, ph

                def stage_gru1(t):
                    r = wp.tile([128, HS], f32, tag=f"r{t}", name=f"r{v}_{t}")
                    z = wp.tile([128, HS], f32, tag=f"z{t}", name=f"z{v}_{t}")
                    nc.scalar.activation(r[:, :], rzr[t][:, :HS], AF.Sigmoid)
                    nc.scalar.activation(z[:, :], rzz[t][:, :HS], AF.Sigmoid)
                    tm = wp.tile([128, HS], f32, tag=f"tmp{t}", name=f"tm{v}_{t}")
                    nc.vector.tensor_tensor(tm[:, :], r[:, :], hnp[t][:, :HS],
                                            OP.mult)
                    nc.vector.tensor_tensor(tm[:, :], tm[:, :], inp[t][:, :HS],
                                            OP.add)
                    n = wp.tile([128, HS], f32, tag=f"n{t}", name=f"n{v}_{t}")
                    nc.scalar.activation(n[:, :], tm[:, :], AF.Tanh)
                    r_[t], z_[t], tmp_[t], n_[t] = r, z, tm, n

                def stage_gru2(t):
                    # h = z*(A_v - n) + n   (tile0 on DVE, tile1 on GPSIMD)
                    ge = nc.vector if t == 0 else nc.gpsimd
                    tm, n, z = tmp_[t], n_[t], z_[t]
                    ge.tensor_tensor(tm[:, :], A[v][t][:, :HS], n[:, :], OP.subtract)
                    ge.tensor_tensor(tm[:, :], tm[:, :], z[:, :], OP.mult)
                    ge.tensor_tensor(Ht[t][:, :HS], tm[:, :], n[:, :], OP.add)

                def stage_th(t):
                    th = pp.tile([128, 512], bf16, tag="ps", name=f"th{v}_{t}")
                    for i, (o, wd) in enumerate(CH):
                        nc.tensor.transpose(th[:wd, i * 128:(i + 1) * 128],
                                            Ht[t][:, o:o + wd], IDB[:, :])
                    ht = hip.tile([128, 512], f32r, tag=f"ht{t}",
                                  name=f"ht{v}_{t}")
                    nc.scalar.copy(ht[:, :], th[:, :])
                    htt[t] = ht
                    if v == MAXN - 1:
                        HT_final[t] = ht

                def stage_gated_mm(t):
                    zp = pp.tile([128, 512], f32, tag="ps", name=f"zp{v}_{t}")
                    mp = pp.tile([128, 512], f32, tag="ps", name=f"mp{v}_{t}")
                    vsel = EYE[:, v:v + 1].broadcast_to([MAXN, 128])
                    for i, (o, wd) in enumerate(CH):
                        hsl = htt[t][0:wd, i * 128:(i + 1) * 128]
                        mm(zp[:, :HS2], hsl, WG[i][:, :], start=(i == 0), stop=False)
                        mm(mp[:, :HS2], hsl, WM[i][:, :], start=(i == 0), stop=False)
                    mm(zp[:, :HS2], vsel, WGV[:, :], start=False, stop=True)
                    mm(mp[:, :HS2], vsel, WMV[:, :], start=False, stop=True)
                    zpp[t], mpp[t] = zp, mp

                def stage_g(t):
                    sg = wp.tile([128, HS2], f32, tag=f"sg{t}", name=f"sg{v}_{t}")
                    nc.scalar.activation(sg[:, :], zpp[t][:, :HS2], AF.Sigmoid)
                    # mp col 501 is 0 -> G col 501 stays 0
                    nc.vector.tensor_tensor(Gt[t][:, :], sg[:, :],
                                            mpp[t][:, :HS2], OP.mult)

                def stage_updates():
                    for w in range(v + 1, MAXN):
                        for t in range(2):
                            nc.vector.scalar_tensor_tensor(
                                A[w][t][:, :], Gt[t][:, :],
                                ADJG[t][:, v * MAXN + w:v * MAXN + w + 1],
                                A[w][t][:, :], OP.mult, OP.add)

                # emission order = scheduler priority; stagger the two tiles
                stage_in(0); stage_in(1)
                stage_ta(0); stage_gates(0)
                stage_ta(1); stage_gates(1)
                stage_gru1(0); stage_gru2(0)
                stage_th(0)
                stage_gru1(1); stage_gru2(1)
                if v < MAXN - 1:
                    stage_gated_mm(0)
                    stage_th(1)
                    stage_gated_mm(1)
                    stage_g(0); stage_g(1)
                    stage_updates()
                else:
                    stage_th(1)

            # ---- readout ----
            for t in range(2):
                op = pp.tile([128, 512], f32, tag="ps", name=f"op{t}")
                for i, (o, wd) in enumerate(CH):
                    hsl = HT_final[t][0:wd, i * 128:(i + 1) * 128]
                    mm(op[:, :2 * NZ], hsl, W12[i][:, :],
                       start=(i == 0), stop=(i == 3))
                ob = wp.tile([128, 2 * NZ], f32, tag=f"ob{t}", name=f"ob{t}")
                nc.scalar.copy(ob[:, :], op[:, :2 * NZ])
                nc.sync.dma_start(d_out[t * 128:(t + 1) * 128, :], ob[:, :])

    nc.compile()
    return nc


def _host_prep(types, feats, adj, Wg, bg, Wm, W_ih, b_ih, W_hh, b_hh, W1, b1, W2, b2):
    """Build per-core input maps (numpy only)."""
    f = np.float32
    types = np.asarray(types).astype(np.int64)
    feats = np.asarray(feats, dtype=f)
    adj = np.asarray(adj, dtype=f)
    Wg, bg, Wm = np.asarray(Wg, f), np.asarray(bg, f), np.asarray(Wm, f)
    W_ih, b_ih = np.asarray(W_ih, f), np.asarray(b_ih, f)
    W_hh, b_hh = np.asarray(W_hh, f), np.asarray(b_hh, f)
    W1, b1 = np.asarray(W1, f), np.asarray(b1, f)
    W2, b2 = np.asarray(W2, f), np.asarray(b2, f)

    bsz = types.shape[0]
    bs = bsz // NCORES

    # X^T with ones row: [48, MAXN*bs] per core
    X = np.zeros((bsz, MAXN, XDIM + 1), dtype=f)
    onehot = np.eye(NVT_EFF, dtype=f)[types.reshape(-1) % NVT_EFF]
    X[:, :, :NVT_EFF] = onehot.reshape(bsz, MAXN, NVT_EFF)
    X[:, :, NVT_EFF] = feats
    X[:, :, XDIM] = 1.0

    # constant gated vectors c_u for zero hidden state
    zg = 1.0 / (1.0 + np.exp(-(bg[None, :] + Wg[:, HS:].T)))   # [20, 501]
    C = (zg * Wm[:, HS:].T).astype(f)

    def aug(wT, brow):
        return np.concatenate([wT, brow[None, :]], axis=0).astype(f)

    def pad_rz(a):          # [s, 1002] -> [s, 1004] with per-gate 502 halves
        o = np.zeros((a.shape[0], 2 * HS2), dtype=f)
        o[:, :HS] = a[:, :HS]
        o[:, HS2:HS2 + HS] = a[:, HS:]
        return o

    def pad_h(a):           # [s, 501] -> [s, 502]
        o = np.zeros((a.shape[0], HS2), dtype=f)
        o[:, :HS] = a
        return o

    wrzh = pad_rz(aug(W_hh[:RZ].T, b_hh[:RZ]))
    whn = pad_h(aug(W_hh[RZ:].T, b_hh[RZ:]))
    wrzx = pad_rz(aug(W_ih[:RZ].T, b_ih[:RZ]))
    wxnc = np.zeros((84, HS2), dtype=f)
    wxnc[:XDIM + 1] = pad_h(aug(W_ih[RZ:].T, b_ih[RZ:]))
    wxnc[64:84] = pad_h(C)
    wg = pad_h(np.concatenate([Wg[:, :HS].T, bg[None, :]], axis=0).astype(f))
    wgv = pad_h(np.ascontiguousarray(Wg[:, HS:].T))
    wm = pad_h(np.concatenate([Wm[:, :HS].T, np.zeros((1, HS), f)], axis=0))
    wmv = pad_h(np.ascontiguousarray(Wm[:, HS:].T))
    eye20 = np.eye(MAXN, dtype=f)
    w12 = np.concatenate([np.concatenate([W1.T, W2.T], axis=1),
                          np.concatenate([b1, b2])[None, :]], axis=0).astype(f)
    # bf16 identity packed into fp32 cells (2 bf16 per cell)
    identb = np.zeros((128, 64), dtype=np.uint32)
    eye128 = np.eye(128, dtype=np.float32)
    b16 = (eye128.view(np.uint32) >> 16).astype(np.uint32)  # bf16 bits
    identb = (b16[:, 1::2] << 16) | b16[:, 0::2]
    identb = identb.view(f)

    ents, ncols, _ = _pack_layout()

    def place(pack, name, arr):
        r0, nr, c0, ncl = ents[name]
        assert arr.shape == (nr, ncl), (name, arr.shape, (nr, ncl))
        pack[r0:r0 + nr, c0:c0 + ncl] = arr

    umask = (np.arange(MAXN)[:, None] >= np.arange(MAXN)[None, :]).astype(f)

    in_maps = []
    for c in range(NCORES):
        slc = slice(c * bs, (c + 1) * bs)
        Xc = X[slc]                                   # [bs, 20, 48]
        xt = Xc.transpose(2, 1, 0).reshape(XDIM + 1, MAXN * bs)
        adjc = adj[slc]                               # [bs, 20, 20]
        # adjT[u, v*bs+b] = adj[b,u,v], zeroed where u < v (only u>=v used)
        adjm = adjc.transpose(1, 2, 0) * umask[:, :, None]
        pk = np.zeros((84, MAXN * bs), dtype=f)
        pk[:XDIM + 1] = xt
        pk[64:84] = adjm.reshape(MAXN, MAXN * bs)

        pack = np.zeros((128, ncols), dtype=f)
        place(pack, "pk", pk)
        place(pack, "wxnc", wxnc)
        place(pack, "identb", identb)
        for i, (o, s) in enumerate(CH):
            place(pack, f"wrzh{i}", wrzh[o:o + s])
            place(pack, f"whn{i}", whn[o:o + s])
            place(pack, f"w12{i}", w12[o:o + s])
            place(pack, f"wg{i}", wg[o:o + s])
            place(pack, f"wm{i}", wm[o:o + s])
        place(pack, "wrzx", wrzx)
        place(pack, "wgv", wgv)
        place(pack, "wmv", wmv)
        place(pack, "eye20", eye20)
        adjg = adjc.reshape(bs, MAXN * MAXN)
        place(pack, "adjg0", adjg[:128])
        place(pack, "adjg1", adjg[128:])
        in_maps.append(dict(wpack=pack))
    return in_maps


def _get_prog():
    global _PROG
    if _PROG is None:
        _PROG = _build_program()
    return _PROG


def kernel(**inputs):
    from concourse.bass_utils import run_bass_kernel_spmd
    nc = _get_prog()
    in_maps = _host_prep(**inputs)
    res = run_bass_kernel_spmd(nc, in_maps, core_ids=list(range(NCORES)))
    out = np.concatenate([r["out"] for r in res.results], axis=0)
    mu = np.ascontiguousarray(out[:, :NZ])
    logvar = np.ascontiguousarray(out[:, NZ:])
    return mu, logvar
